# revision 36
# baseline (speedup 1.0000x reference)
"""Multi-head self-attention with RoPE on 8 TRN2 NeuronCores.

Sharding: data-parallel over batch (2) x tensor-parallel over heads (4 groups
of 4 heads). Core c handles batch c//4, head group c%4. Each core computes
QKV projections for its 4 heads, RoPE, causal flash attention (transposed
scores layout), and its partial Wo projection. Host sums the 4 partials per
batch.

Shapes (hardcoded): B=2, S=2048, D=1024, H=16, dk=64.
"""

import numpy as np

import concourse.bass as bass
import concourse.bacc as bacc
import concourse.mybir as mybir
import concourse.tile as tile
from concourse import bass_utils
from concourse.alu_op_type import AluOpType

B, S, D, H = 2, 2048, 1024, 16
GH = 4                 # head groups (tensor-parallel degree)
HPG = H // GH          # 4 heads per group
DK = D // H            # 64
DG = D // GH           # 256 local out dims per group
NB = 4                 # q blocks
QB = S // NB           # 512
NKC = S // 128         # 16 k-chunks of 128
NPAIR = HPG // 2       # 2 head pairs per group
SCALE = 1.0 / (DK ** 0.5)

f32 = mybir.dt.float32
f32r = mybir.dt.float32r
AF = mybir.ActivationFunctionType

_CACHE = {}


def _build_program():
    nc = bacc.Bacc("TRN2", target_bir_lowering=False, debug=False)

    xT = nc.dram_tensor("xT", [D, S], f32r, kind="ExternalInput")
    wq = nc.dram_tensor("wq", [D, DG], f32r, kind="ExternalInput")
    wk = nc.dram_tensor("wk", [D, DG], f32r, kind="ExternalInput")
    wv = nc.dram_tensor("wv", [D, DG], f32r, kind="ExternalInput")
    wo = nc.dram_tensor("wo", [DG, D], f32r, kind="ExternalInput")
    c2 = nc.dram_tensor("c2", [128, S], f32, kind="ExternalInput")
    sn2 = nc.dram_tensor("sn2", [128, S], f32, kind="ExternalInput")
    y = nc.dram_tensor("y", [S, D], f32, kind="ExternalOutput")

    with tile.TileContext(nc) as tc:
        _kernel_body(tc, xT, wq, wk, wv, wo, c2, sn2, y)
    nc.compile()
    return nc


def _kernel_body(tc, xT, wq, wk, wv, wo, c2, sn2, y):
    nc = tc.nc
    with (
        tc.tile_pool(name="const", bufs=1) as constp,
        tc.tile_pool(name="xq", bufs=2) as xqp,
        tc.tile_pool(name="kt", bufs=1) as ktp,
        tc.tile_pool(name="qt", bufs=6) as qtp,
        tc.tile_pool(name="vt", bufs=1) as vtp,
        tc.tile_pool(name="rope", bufs=4) as ropep,
        tc.tile_pool(name="pt", bufs=4) as ptp,
        tc.tile_pool(name="on", bufs=4) as onp,
        tc.tile_pool(name="rl", bufs=4) as rlp,
        tc.tile_pool(name="osb", bufs=2) as osbp,
        tc.tile_pool(name="mm_ps", bufs=2, space="PSUM") as mmps,
        tc.tile_pool(name="st_ps", bufs=2, space="PSUM") as stps,
        tc.tile_pool(name="ot_ps", bufs=2, space="PSUM") as otps,
    ):
        # --- constants ---
        wq_sb = constp.tile([128, 8, DG], f32r, tag="wq")
        wk_sb = constp.tile([128, 8, DG], f32r, tag="wk")
        wv_sb = constp.tile([128, 8, DG], f32r, tag="wv")
        wo_sb = constp.tile([128, 2, D], f32r, tag="wo")
        c2_sb = constp.tile([128, S], f32, tag="c2")
        sn2_sb = constp.tile([128, S], f32, tag="sn2")
        nc.sync.dma_start(wq_sb[:], wq.ap().rearrange("(c p) e -> p c e", p=128))
        nc.sync.dma_start(wk_sb[:], wk.ap().rearrange("(c p) e -> p c e", p=128))
        nc.sync.dma_start(c2_sb[:], c2.ap())
        nc.sync.dma_start(sn2_sb[:], sn2.ap())
        nc.sync.dma_start(wv_sb[:], wv.ap().rearrange("(c p) e -> p c e", p=128))
        nc.sync.dma_start(wo_sb[:], wo.ap().rearrange("(c p) e -> p c e", p=128))

        # PE warm-up: ~25 matmuls on zeros so HAM reaches full clock while
        # the constant DMAs are in flight; result lands in a y corner that the
        # block-0 output DMA fully overwrites later.
        with tc.tile_pool(name="warm", bufs=1) as warmp:
            wz = warmp.tile([128, 528], f32r, tag="wz")
            wzo = warmp.tile([16, 16], f32, tag="wzo")
            wps = mmps.tile([16, 512], f32, tag="mm", name="warm_ps")
            nc.gpsimd.memset(wz[:].bitcast(f32), 0.0)
            for i in range(25):
                nc.tensor.matmul(wps[:], wz[:, 512:528], wz[:, 0:512],
                                 start=True, stop=True)
            nc.vector.tensor_copy(wzo[:], wps[:, 0:16])
            nc.sync.dma_start(y.ap()[0:16, 0:16], wzo[:])

        # persistent K (rope'd, transposed) and V tiles
        kt_t = [[ktp.tile([128, QB], f32r, tag=f"kt{p}_{j}", name=f"kt{p}_{j}")
                 for j in range(NB)] for p in range(NPAIR)]
        # V natural layout per k-tile of 128 positions: [128, 4*65]
        # (4 heads x (64 dims + ones column))
        vt_t = [vtp.tile([128, HPG * (DK + 1)], f32r, tag=f"vt{t}", name=f"vt{t}")
                for t in range(NKC)]
        for t in range(NKC):
            vv = vt_t[t][:].rearrange("p (h e) -> p h e", h=HPG)
            nc.gpsimd.memset(vv[:, :, DK:DK + 1].bitcast(f32), 1.0)

        xq_tiles = {}

        def xq_dma(j):
            q0 = j * QB
            xq = xqp.tile([128, 8, QB], f32r, tag="xq", name=f"xq{j}")
            xTv = xT.ap().rearrange("(c p) s -> p c s", p=128)
            for c in range(8):
                nc.scalar.dma_start(xq[:, c, :], xTv[:, c, q0:q0 + QB])
            xq_tiles[j] = xq

        def proj_gen(j, qt_t_out):
            """Generator emitting block j's projections; yields every ~2 MMs."""
            q0 = j * QB
            xq = xq_tiles.pop(j)
            for p in range(NPAIR):
                for which in ("q", "k"):
                    e0 = p * 128
                    w_sb = wq_sb if which == "q" else wk_sb
                    ps = mmps.tile([128, QB], f32, tag="mm",
                                   name=f"{which}ps{j}_{p}")
                    for c in range(8):
                        nc.tensor.matmul(ps[:], w_sb[:, c, e0:e0 + 128],
                                         xq[:, c, :], start=(c == 0), stop=(c == 7))
                        if c % 2 == 1:
                            yield
                    if which == "q":
                        qt = qtp.tile([128, QB], f32r, tag="qt", name=f"qt{j}_{p}")
                        _rope(nc, tc, ropep, qt, ps, c2_sb, sn2_sb, q0)
                        qt_t_out[p] = qt
                    else:
                        _rope(nc, tc, ropep, kt_t[p][j], ps, c2_sb, sn2_sb, q0)
                    yield
            for t in range(4):
                v_ps = mmps.tile([128, DG], f32, tag="mm", name=f"vps{j}_{t}")
                for c in range(8):
                    nc.tensor.matmul(v_ps[:], xq[:, c, 128 * t:128 * t + 128],
                                     wv_sb[:, c, :], start=(c == 0), stop=(c == 7))
                    if c % 2 == 1:
                        yield
                vv = vt_t[4 * j + t][:].rearrange("p (h e) -> p h e", h=HPG)
                nc.vector.tensor_copy(
                    out=vv[:, :, 0:DK],
                    in_=v_ps[:].rearrange("p (h e) -> p h e", h=HPG))
                yield

        def final_gen(j, on_tiles, epilogue=False):
            """Generator emitting block j's output projection; yields per unit.
            In the epilogue ACT is idle: use it for evacuations, and spread the
            output DMAs across both queues."""
            q0 = j * QB
            for t in range(4):
                osb = osbp.tile([128, D], f32, tag="osb", name=f"osb{j}_{t}")
                for eb in range(2):
                    o_ps = mmps.tile([128, 512], f32, tag="mm",
                                     name=f"ops{j}_{t}_{eb}")
                    for c in range(2):
                        nc.tensor.matmul(
                            o_ps[:], on_tiles[c][:, 128 * t:128 * t + 128],
                            wo_sb[:, c, 512 * eb:512 * eb + 512],
                            start=(c == 0), stop=(c == 1))
                    if epilogue:
                        nc.scalar.copy(osb[:, 512 * eb:512 * eb + 512], o_ps[:])
                    else:
                        nc.vector.tensor_copy(osb[:, 512 * eb:512 * eb + 512],
                                              o_ps[:])
                    yield
                qeng = nc.scalar if (epilogue and t % 2) else nc.sync
                qeng.dma_start(
                    y.ap()[q0 + 128 * t:q0 + 128 * t + 128, :], osb[:])
                yield

        # prologue: block 0 fully projected up front; block 1's x prefetched
        xq_dma(0)
        xq_dma(1)
        qt_blocks = {0: [None] * NPAIR}
        for _ in proj_gen(0, qt_blocks[0]):
            pass

        on_prev = None
        for j in range(NB):
            q0 = j * QB
            if j + 2 < NB:
                xq_dma(j + 2)  # prefetch a full block ahead
            fillers = []
            if j + 1 < NB:
                qt_blocks[j + 1] = [None] * NPAIR
                g = proj_gen(j + 1, qt_blocks[j + 1])
                next(g)
                fillers.append(g)
            if on_prev is not None:
                fillers.append(final_gen(j - 1, on_prev))
            qt_t = qt_blocks.pop(j)
            total_chunks = 2 * 4 * (j + 1)
            # steps available: proj has 48 yields, final has 8
            total_steps = (48 if j + 1 < NB else 0) + (8 if on_prev is not None else 0)
            done_steps = 0
            chunk_i = 0

            def drain_chains(frac):
                nonlocal done_steps, fillers
                want = min(total_steps, int(total_steps * frac + 0.999))
                while done_steps < want and fillers:
                    try:
                        next(fillers[0])
                        done_steps += 1
                    except StopIteration:
                        fillers.pop(0)

            # --- attention, one head pair at a time ---
            nkc = 4 * (j + 1)
            on_tiles = [onp.tile([128, QB], f32r, tag=f"on{p}", name=f"on{p}_{j}")
                        for p in range(NPAIR)]
            deferred_norm = []
            for p in range(NPAIR):
                ot_h = [otps.tile([128, QB], f32, tag="ot", name=f"ot{j}_{p}_{hh}")
                        for hh in range(2)]
                for kc in range(nkc):
                    kt_tile = kt_t[p][kc // 4]
                    koff = 128 * (kc % 4)
                    r = 128 * kc - q0  # diagonal offset; >=0 means diagonal tile
                    # wide 2-bank tiles: head hh occupies cols [512hh:512hh+512]
                    st = stps.tile([128, 2 * QB], f32, tag="st")
                    pt = ptp.tile([128, 2 * QB], f32r, tag="pt")
                    v0 = max(r, 0)  # first valid q column in this chunk
                    for hh in range(2):
                        pb = 64 * hh
                        nc.tensor.matmul(
                            st[:, QB * hh + v0:QB * hh + QB],
                            kt_tile[pb:pb + 64, koff:koff + 128],
                            qt_t[p][pb:pb + 64, v0:], start=True, stop=True,
                            tile_position=(pb, 0))
                    chunk_i += 1
                    if total_steps:
                        drain_chains(chunk_i / total_chunks)
                    if r > 0:
                        stv = st[:].rearrange("p (h q) -> p h q", h=2)
                        ptv = pt[:].rearrange("p (h q) -> p h q", h=2)
                        nc.scalar.activation(ptv[:, :, r:], stv[:, :, r:],
                                             AF.Exp, scale=SCALE)
                        nc.gpsimd.memset(ptv[:, :, 0:r].bitcast(f32), 0.0)
                    else:
                        nc.scalar.activation(pt[:], st[:], AF.Exp, scale=SCALE)
                    if r >= 0:
                        # causal triangle: keep col i >= partition kk
                        ptv = pt[:].rearrange("p (h q) -> p h q", h=2)
                        nc.gpsimd.affine_select(
                            out=ptv[:, :, r:r + 128], in_=ptv[:, :, r:r + 128],
                            pattern=[[0, 2], [1, 128]],
                            compare_op=AluOpType.is_ge, fill=0.0,
                            base=0, channel_multiplier=-1)
                    for hh in range(2):
                        h = 2 * p + hh
                        vv = vt_t[kc][:].rearrange("p (h e) -> p h e", h=HPG)
                        nc.tensor.matmul(
                            ot_h[hh][0:DK + 1, v0:], vv[:, h, :],
                            pt[:, QB * hh + v0:QB * hh + QB],
                            start=(kc == 0), stop=(kc == nkc - 1))
                    if kc == 0 and deferred_norm:
                        for fn in deferred_norm:
                            fn()
                        deferred_norm = []

                # normalize: o /= l, write into oTn chunk tile [128, 512]
                def make_norm(p, ot_h):
                    def emit():
                        for hh in range(2):
                            rl = rlp.tile([1, QB], f32, tag="rl",
                                          name=f"rl{j}_{p}_{hh}")
                            rlb = rlp.tile([DK, QB], f32, tag="rlb",
                                           name=f"rlb{j}_{p}_{hh}")
                            nc.vector.tensor_copy(rl[:], ot_h[hh][DK:DK + 1, :])
                            nc.vector.reciprocal_approx_fast(rl[:], rl[:])
                            nc.gpsimd.partition_broadcast(rlb[:], rl[:])
                            nc.vector.tensor_tensor(
                                out=on_tiles[p][64 * hh:64 * hh + 64, :],
                                in0=ot_h[hh][0:DK, :],
                                in1=rlb[:],
                                op=AluOpType.mult)
                    return emit
                deferred_norm.append(make_norm(p, ot_h))
            for fn in deferred_norm:
                fn()

            # finish any remaining filler work for this window
            for g in fillers:
                for _ in g:
                    pass
            on_prev = on_tiles

        # epilogue: last block's output projection
        for _ in final_gen(NB - 1, on_prev, epilogue=True):
            pass


_SWAP_MASK = [i ^ 1 for i in range(32)]


def _rope(nc, tc, ropep, out_t, in_ps, c2_sb, sn2_sb, q0):
    """out = c2 * in + sn2 * swap(in), swap = exchange even/odd partitions."""
    t1 = ropep.tile([128, QB], f32, tag="t1")
    t2 = ropep.tile([128, QB], f32, tag="t2")
    tmp = ropep.tile([128, QB], f32, tag="tmp")
    nc.vector.tensor_tensor(out=t1[:], in0=in_ps[:], in1=c2_sb[:, q0:q0 + QB],
                            op=AluOpType.mult)
    nc.vector.stream_shuffle(tmp[:], in_ps[:], _SWAP_MASK)
    nc.gpsimd.tensor_tensor(out=t2[:], in0=tmp[:], in1=sn2_sb[:, q0:q0 + QB],
                            op=AluOpType.mult)
    nc.vector.tensor_tensor(out=out_t[:], in0=t1[:], in1=t2[:], op=AluOpType.add)


def _prep_inputs(x, Wq, Wk, Wv, Wo, cos, sin):
    """Build per-core input maps."""
    dk2 = DK // 2
    # c2[p, s] = cos[s, (p%64)//2]; sn2[p,s] = -/+ sin[s, (p%64)//2]
    idx = (np.arange(128) % 64) // 2
    sign = np.where(np.arange(128) % 2 == 0, -1.0, 1.0).astype(np.float32)
    c2 = np.ascontiguousarray(cos.T[idx, :])                      # [128, S]
    sn2 = np.ascontiguousarray(sin.T[idx, :] * sign[:, None])     # [128, S]

    xTs = [np.ascontiguousarray(x[b].T) for b in range(B)]
    in_maps = []
    for core in range(8):
        b, g = core // GH, core % GH
        sl = slice(DG * g, DG * (g + 1))
        in_maps.append({
            "xT": xTs[b],
            "wq": np.ascontiguousarray(Wq[sl, :].T),
            "wk": np.ascontiguousarray(Wk[sl, :].T),
            "wv": np.ascontiguousarray(Wv[sl, :].T),
            "wo": np.ascontiguousarray(Wo[:, sl].T),
            "c2": c2,
            "sn2": sn2,
        })
    return in_maps


def _install_profile_hook():
    """Shim antenv.axon_hooks so run_bass_kernel_spmd(trace=True) can reach
    the axon NTFF profiler (the agent image's antenv lacks axon_hooks)."""
    import sys
    import types
    try:
        from antenv.axon_hooks import get_axon_ntff_profile_hook  # noqa: F401
        return
    except ImportError:
        pass
    try:
        import antenv
        from trn_agent_boot.trn_boot import _ntff_profile_via_ctypes
        mod = types.ModuleType("antenv.axon_hooks")
        mod._hook = None

        def set_axon_ntff_profile_hook(h):
            mod._hook = h

        def get_axon_ntff_profile_hook():
            return mod._hook

        mod.set_axon_ntff_profile_hook = set_axon_ntff_profile_hook
        mod.get_axon_ntff_profile_hook = get_axon_ntff_profile_hook
        sys.modules["antenv.axon_hooks"] = mod
        antenv.axon_hooks = mod
        hook = _ntff_profile_via_ctypes("/opt/axon/libaxon_pjrt.so")
        if hook is not None:
            set_axon_ntff_profile_hook(hook)
    except Exception as e:  # profiling is best-effort
        print(f"profile hook install failed: {e}")


def kernel(x, Wq, Wk, Wv, Wo, cos, sin, _trace=False):
    x = np.asarray(x, dtype=np.float32)
    Wq = np.asarray(Wq, dtype=np.float32)
    Wk = np.asarray(Wk, dtype=np.float32)
    Wv = np.asarray(Wv, dtype=np.float32)
    Wo = np.asarray(Wo, dtype=np.float32)
    cos = np.asarray(cos, dtype=np.float32)
    sin = np.asarray(sin, dtype=np.float32)

    if "nc" not in _CACHE:
        _CACHE["nc"] = _build_program()
    nc = _CACHE["nc"]

    in_maps = _prep_inputs(x, Wq, Wk, Wv, Wo, cos, sin)
    if _trace:
        _install_profile_hook()
    res = bass_utils.run_bass_kernel_spmd(
        nc, in_maps, core_ids=list(range(8)), trace=_trace)
    if _trace:
        _CACHE["last_result"] = res

    out = np.empty((B, S, D), dtype=np.float32)
    for b in range(B):
        acc = res.results[4 * b]["y"].astype(np.float32)
        for g in range(1, GH):
            acc = acc + res.results[4 * b + g]["y"]
        out[b] = acc
    return out
